# revision 31
# baseline (speedup 1.0000x reference)
"""Trainium2 Bass kernel for nn_CNL_5 (5-scale context non-local block).

Sharding: 8 cores = 4 samples x 2 query-subsets. Local query column order is
L = 64*j + q  (q = z-conv input channel = n//36-block, j = n%18), chosen so the
xbar DMA transpose (out[p,b,c] = in[c,128b+p]) directly yields the z-conv
operand x[q, pixel] with j-parity split across partition halves. outT is
padded to 640-col halves so each query-half transposes independently.

BN batch stats travel as per-channel (s1, s2) quadratic forms [128,20] through
one small AllGather; heavy math is fp16/bf16 on the PE at 1 cyc/row.

v2: unit-queue scheduling — conv/gram/svec/stat PE work is split into ~250ns
units interleaved between score and A@g matmuls so the Act engine (the
binding resource, exp at 0.833ns/col + 185ns/inst) never starves at half or
scale boundaries; op PSUM is evacuated to SBUF by one DVE copy so the single
op bank recycles in ~0.7us; DMAs are merged into blobs (HWDGE descriptor cost
is 625ns each, serial); PE warmup matmuls burn the 3us pstate ramp during the
initial DMA wait; one xbar transpose per scale.
"""
import numpy as np
import ml_dtypes
from contextlib import ExitStack

import concourse.bass as bass
import concourse.bacc as bacc
import concourse.tile as tile
from concourse import mybir
from concourse import bass_utils
from concourse.alu_op_type import AluOpType

F32 = mybir.dt.float32
F16 = mybir.dt.float16
BF16 = mybir.dt.bfloat16
AFT = mybir.ActivationFunctionType
AXX = mybir.AxisListType.X

NCORES = 8
CP = 256
QL = 1152
CR = [64, 256, 512, 1024, 2048]
MS = [2304, 2304, 576, 144, 36]
CSH = [0.0, 10.0, 15.0, 25.0, 40.0]
EPS = 1e-5
NPIX = 4 * 2304.0
SCHED = [0, 3, 4, 2, 1]
KT = [max(1, c // 128) for c in CR]

# wb16 blob layout (f16): twt | pwt_s0 | gwt_s0 | pwt_rest | gwt_rest |
# zwt16 | i128
W_TWT = 0
W_PWT0 = 128
W_GWT0 = 192
W_PWTR = 256
W_GWTR = W_PWTR + 64 * (sum(KT) - KT[0])
W_ZWT = W_GWTR + 64 * (sum(KT) - KT[0])
W_I128 = W_ZWT + 256
W_END = W_I128 + 128
POFF = {0: W_PWT0}
GOFF = {0: W_GWT0}
_off = 0
for _s in range(1, 5):
    POFF[_s] = W_PWTR + _off
    GOFF[_s] = W_GWTR + _off
    _off += 64 * KT[_s]

# wb32 blob layout (f32): zwtf | zw65 | gmp
Z_ZWTF = 0
Z_ZW65 = 256
Z_GMP = Z_ZW65 + 650
Z_END = Z_GMP + 13

_CACHED = {}


def mtiles(M):
    out, off = [], 0
    while off < M:
        w = min(128, M - off)
        out.append((off, w))
        off += w
    return out


def chunks512(N):
    out, off = [], 0
    while off < N:
        w = min(512, N - off)
        out.append((off, w))
        off += w
    return out


def build():
    nc = bacc.Bacc("TRN2", target_bir_lowering=False, debug=False,
                   num_devices=NCORES)
    persp_d = nc.dram_tensor("persp", [128, 2 * QL], F16,
                             kind="ExternalInput").ap()
    wb16_d = nc.dram_tensor("wb16", [128, W_END], F16,
                            kind="ExternalInput").ap()
    wb32_d = nc.dram_tensor("wb32", [128, Z_END], F32,
                            kind="ExternalInput").ap()
    resp0_d = nc.dram_tensor("resp0", [64, MS[0]], F16,
                             kind="ExternalInput").ap()
    resp1_d = nc.dram_tensor("resp1", [128, KT[1] * MS[1]], F16,
                             kind="ExternalInput").ap()
    r234_d = nc.dram_tensor("r234", [128, KT[2] * MS[2] + KT[3] * MS[3] +
                                     KT[4] * MS[4]], F16,
                            kind="ExternalInput").ap()
    out_d = nc.dram_tensor("out", [128, 2 * QL], F32, kind="ExternalOutput").ap()

    with tile.TileContext(nc) as tc, ExitStack() as ctx:
        sb = ctx.enter_context(tc.tile_pool(name="sb", bufs=1))
        p2 = ctx.enter_context(tc.tile_pool(name="p2", bufs=2))
        et3 = ctx.enter_context(tc.tile_pool(name="et3", bufs=32))
        p16p = ctx.enter_context(tc.tile_pool(name="p16p", bufs=5))
        dram = ctx.enter_context(tc.tile_pool(name="dram", bufs=1, space="DRAM"))
        psc = ctx.enter_context(tc.tile_pool(name="psc", bufs=2, space="PSUM"))
        pop = ctx.enter_context(tc.tile_pool(name="pop", bufs=1, space="PSUM"))
        pcv = ctx.enter_context(tc.tile_pool(name="pcv", bufs=1, space="PSUM"))
        pgx = ctx.enter_context(tc.tile_pool(name="pgx", bufs=1, space="PSUM"))

        # ---------------- DMA: critical pieces first ----------------
        wb16 = sb.tile([128, W_END], F16, tag="wb16", name="wb16")
        nc.sync.dma_start(wb16[:, 0:W_PWTR], wb16_d[:, 0:W_PWTR])
        resp_sb = [None] * 5
        r0 = sb.tile([64, MS[0]], F16, tag="resp0", name="resp0")
        nc.sync.dma_start(r0[:], resp0_d)
        resp_sb[0] = r0
        persp_sb = sb.tile([128, 2 * QL], F16, tag="persp", name="persp")
        pv_sb = persp_sb[:].rearrange("p (k q) -> p k q", k=2)
        pv_d = persp_d.rearrange("p (k q) -> p k q", k=2)
        nc.sync.dma_start(pv_sb[:, :, 0:576], pv_d[:, :, 0:576])
        nc.sync.dma_start(pv_sb[:, :, 576:1152], pv_d[:, :, 576:1152])
        r234 = sb.tile([128, KT[2] * MS[2] + KT[3] * MS[3] + KT[4] * MS[4]],
                       F16, tag="r234", name="r234")
        nc.sync.dma_start(r234[:], r234_d)
        resp_sb[2] = r234[:, 0:KT[2] * MS[2]]
        resp_sb[3] = r234[:, KT[2] * MS[2]:KT[2] * MS[2] + KT[3] * MS[3]]
        resp_sb[4] = r234[:, KT[2] * MS[2] + KT[3] * MS[3]:]
        nc.sync.dma_start(wb16[:, W_PWTR:W_END], wb16_d[:, W_PWTR:W_END])
        wb32 = sb.tile([128, Z_END], F32, tag="wb32", name="wb32")
        nc.sync.dma_start(wb32[:], wb32_d)
        r1 = sb.tile([128, KT[1] * MS[1]], F16, tag="resp1", name="resp1")
        nc.sync.dma_start(r1[:], resp1_d)
        resp_sb[1] = r1

        twt_sb = wb16[:, W_TWT:W_TWT + 128]
        zwt16_sb = wb16[:, W_ZWT:W_ZWT + 256]
        i128_sb = wb16[:, W_I128:W_I128 + 128]
        zwtf_sb = bass.AP(tensor=wb32[:].tensor, offset=wb32[:].offset + Z_ZWTF,
                          ap=[[wb32[:].ap[0][0], 64], [1, 256]])
        zw65g_sb = wb32[:, Z_ZW65:Z_ZW65 + 650]
        gmp_sb = wb32[:, Z_GMP:Z_GMP + 13]

        # ---------------- constants ----------------
        bias_sb = []
        for s in range(5):
            bt = sb.tile([128, 1], F32, tag=f"bias{s}", name=f"bias{s}")
            nc.vector.memset(bt[:], -CSH[s])
            bias_sb.append(bt)
        ones128 = sb.tile([1, 128], F16, tag="ones128", name="ones128")
        nc.vector.memset(ones128[:], 1.0)
        onesc = sb.tile([64, 1], F16, tag="onesc", name="onesc")
        nc.vector.memset(onesc[:], 1.0)
        g_all = [sb.tile([128, 128 * 18], BF16, tag=f"gall{i}", name=f"gall{i}")
                 for i in range(5)]
        for i in range(5):
            nc.gpsimd.memset(
                g_all[i][:].rearrange("p (k c) -> p k c", c=128)[:, :, 64:128],
                1.0)
        # outT ring: pre-zero the 64-col pads of both ring slots (mult only
        # ever writes cols 0:576 / 640:1216, so pads stay zero across reuse)
        oT = [p2.tile([64, 1280], F16, tag="outT", name=f"outTz{i}")
              for i in range(2)]
        for i in range(2):
            for h in range(2):
                nc.gpsimd.memset(oT[i][:, 640 * h + 576:640 * h + 640], 0.0)

        # ---------------- PE warmup (pstate ramp burn) ----------------
        warm = sb.tile([128, 512], F16, tag="warm", name="warm")
        nc.vector.memset(warm[:], 0.001)
        for wi in range(4):
            wp = pcv.tile([128, 512], F32, tag="cv", name=f"warm{wi}")
            nc.tensor.matmul(wp[:, 0:512], warm[:, 0:128], warm[:, 0:512],
                             start=True, stop=True)

        # ---------------- t conv: t16 [64, QL] ----------------
        # h0 chunks emitted directly; h1 chunks (blocked on the second persp
        # DMA piece) deferred so they don't head-block the PE queue
        t16 = sb.tile([64, QL], F16, tag="t16", name="t16")

        def t_unit(off, w):
            def emit():
                tp = pgx.tile([128, 512], F32, tag="gx", name="gx")
                for kk in range(2):
                    nc.tensor.matmul(
                        tp[0:64, 0:w], twt_sb[:, 64 * kk:64 * kk + 64],
                        persp_sb[:, QL * kk + off:QL * kk + off + w],
                        start=(kk == 0), stop=(kk == 1))
                nc.vector.tensor_copy(t16[:, off:off + w], tp[0:64, 0:w])
            return emit

        for off, w in ((0, 512), (512, 64)):
            t_unit(off, w)()

        # ---------------- deferred PE unit machinery ----------------
        # Units are (key, est_ns, emit_fn) closures with bounded PE work,
        # popped under a per-slot budget between the pipelined score/op
        # matmuls so the PE's idle slack under the Act exp stream absorbs
        # all side work (convs, gram/svec, BN stat partials).
        p16_sb, xw_sb = {}, {}

        def conv_units(s):
            nct = mtiles(CR[s])
            p16 = p16p.tile([64, MS[s]], F16, tag="p16", name=f"p16_{s}")
            p16_sb[s] = p16
            units = []

            def p_unit(off, w, pp_box, klo, khi):
                def emit():
                    if klo == 0:
                        pp_box[0] = pcv.tile([128, 512], F32, tag="cv",
                                             name="cv")
                    pp = pp_box[0]
                    for kk in range(klo, khi):
                        nc.tensor.matmul(
                            pp[0:64, 0:w],
                            wb16[0:nct[kk][1], POFF[s] + 64 * kk:POFF[s] + 64 * kk + 64],
                            resp_sb[s][0:nct[kk][1],
                                       MS[s] * kk + off:MS[s] * kk + off + w],
                            start=(kk == 0), stop=(kk == len(nct) - 1))
                    if khi == len(nct):
                        nc.vector.tensor_copy(p16[:, off:off + w],
                                              pp[0:64, 0:w])
                return (s, (khi - klo) * w * 0.42, emit)

            def g_unit(b0, batch, gp_box, first, last):
                def emit():
                    ga = g_all[SCHED.index(s)]
                    if first:
                        gp_box[0] = pcv.tile([128, 512], F32, tag="cv",
                                             name="cv")
                    gp = gp_box[0]
                    for k, (moff, mw) in enumerate(batch):
                        for kk in range(len(nct)):
                            nc.tensor.matmul(
                                gp[0:mw, 64 * (b0 % 8 + k):64 * (b0 % 8 + k) + 64],
                                resp_sb[s][0:nct[kk][1],
                                           MS[s] * kk + moff:MS[s] * kk + moff + mw],
                                wb16[0:nct[kk][1],
                                     GOFF[s] + 64 * kk:GOFF[s] + 64 * kk + 64],
                                start=(kk == 0), stop=(kk == len(nct) - 1))
                    if last:
                        blo = (b0 // 8) * 8
                        nb = b0 % 8 + len(batch)
                        dst = ga[:].rearrange("p (k c) -> p k c", c=128)[
                            :, blo:blo + nb, 0:64]
                        src = gp[:].rearrange("p (k c) -> p k c", c=64)[
                            :, 0:nb, :]
                        nc.vector.tensor_copy(dst, src)
                return (s, len(batch) * len(nct) * 27, emit)

            for off, w in chunks512(MS[s]):
                pp_box = [None]
                ngrp = max(1, int(np.ceil(len(nct) * w / 560.0)))
                kstep = int(np.ceil(len(nct) / ngrp))
                klo = 0
                while klo < len(nct):
                    khi = min(klo + kstep, len(nct))
                    units.append(p_unit(off, w, pp_box, klo, khi))
                    klo = khi
            mts = mtiles(MS[s])
            mstep = max(1, 8 // len(nct))
            for b0 in range(0, len(mts), 8):
                gp_box = [None]
                grp = mts[b0:b0 + 8]
                for bb in range(0, len(grp), mstep):
                    batch = grp[bb:bb + mstep]
                    units.append(g_unit(b0 + bb, batch, gp_box,
                                        first=(bb == 0),
                                        last=(bb + mstep >= len(grp))))
            return units

        def gram_units(si, s, h, outT, gm_box):
            # gram (64x64) and svec column (col 64) share the gx bank in one
            # accumulation group: the h0/j0 gram start zeroes the bank, the
            # final svec matmul carries stop.
            units = []

            def unit(jlo, jhi):
                def emit():
                    if h == 0 and jlo == 0:
                        gm_box[0] = pgx.tile([128, 512], F32, tag="gx",
                                             name="gm")
                    gm = gm_box[0]
                    for j in range(jlo, jhi):
                        nc.tensor.matmul(
                            gm[0:64, 0:64],
                            outT[:, 640 * h + 64 * j:640 * h + 64 * j + 64],
                            outT[:, 640 * h + 64 * j:640 * h + 64 * j + 64],
                            start=(h == 0 and j == 0), stop=False,
                            skip_group_check=True)
                        nc.tensor.matmul(
                            gm[0:64, 64:65],
                            outT[:, 640 * h + 64 * j:640 * h + 64 * j + 64],
                            onesc[:],
                            start=False, stop=(h == 1 and j == 8),
                            skip_group_check=True)
                return (('g', si), (jhi - jlo) * 29, emit)

            for jlo in range(0, 9, 3):
                units.append(unit(jlo, min(jlo + 3, 9)))
            return units

        def gcat_unit(si, G_cat, gm_box):
            def emit():
                nc.vector.tensor_copy(G_cat[:, 65 * si:65 * si + 65],
                                      gm_box[0][0:64, 0:65])
            return (('g', si), 0, emit)

        def zstat_units(silo, sihi, G_cat, arin_sb):
            # BN quadratic-form partials for scheduled scales silo..sihi-1
            units = []
            c0, c1 = 65 * silo, 65 * sihi
            nsc = sihi - silo
            zg_box = [None, None]

            def mm_unit(t, piece, npieces):
                def emit():
                    if piece == 0:
                        pool, tag = (pgx, "gx") if t == 0 else (pcv, "cv")
                        zg_box[t] = pool.tile([128, 512], F32, tag=tag,
                                              name="zg")
                    w = c1 - c0
                    lo = piece * w // npieces
                    hi = (piece + 1) * w // npieces
                    nc.tensor.matmul(zg_box[t][:, lo:hi],
                                     zwtf_sb[:, 128 * t:128 * t + 128],
                                     G_cat[:, c0 + lo:c0 + hi],
                                     start=(piece == 0),
                                     stop=(piece == npieces - 1),
                                     skip_group_check=True)
                return (('z', silo), (c1 - c0) * 1.7 / npieces, emit)

            def dve_unit(t):
                def emit():
                    zg = zg_box[t]
                    zz = p2.tile([128, c1 - c0], F32, tag="zz", name="zz")
                    nc.vector.tensor_tensor(
                        zz[:], zg[:, 0:c1 - c0],
                        zw65g_sb[:, 325 * t + c0:325 * t + c1],
                        op=AluOpType.mult)
                    s2p = p2.tile([128, nsc], F32, tag="s2p", name="s2p")
                    nc.vector.tensor_reduce(
                        s2p[:],
                        zz[:].rearrange("p (s c) -> p s c", c=65), AXX,
                        AluOpType.add)
                    nc.vector.tensor_copy(
                        arin_sb[:, 10 * t + silo:10 * t + sihi], s2p[:])
                    nc.vector.tensor_copy(
                        arin_sb[:, 10 * t + 5 + silo:10 * t + 5 + sihi],
                        bass.AP(tensor=zg[:].tensor,
                                offset=zg[:].offset + 64,
                                ap=[[zg[:].ap[0][0], 128], [65, nsc]]))
                return (('z', silo), 0, emit)

            npieces = 2 if (c1 - c0) > 130 else 1
            for t in range(2):
                for piece in range(npieces):
                    units.append(mm_unit(t, piece, npieces))
                units.append(dve_unit(t))
            return units

        def pop_units(budget=210.0):
            while pending and budget >= 0:
                key, est, fn = pending[0]
                if est > budget and budget < 210.0:
                    break
                pending.pop(0)
                fn()
                budget -= max(est, 27.0)

        pending = []
        pending += conv_units(SCHED[0])
        pending += [("t2", 240, t_unit(576, 512)), ("t2", 30, t_unit(1088, 64))]
        pending += conv_units(SCHED[1])

        # ---------------- attention ----------------
        arin_sb = sb.tile([128, 20], F16, tag="arin", name="arin")
        G_cat = sb.tile([64, 325], F32, tag="G_cat", name="G_cat")
        gm_boxes = [[None] for _ in range(5)]
        for si, s in enumerate(SCHED):
            mts = mtiles(MS[s])
            nmt = len(mts)
            ga = g_all[si]
            while pending and pending[0][0] == s:
                key, est, fn = pending.pop(0)
                fn()
            if si == 0:
                for sj in range(2, 5):
                    pending += conv_units(SCHED[sj])
            outT = p2.tile([64, 1280], F16, tag="outT", name=f"outT{s}")
            jobs = [(h, k) for h in range(2) for k in range(nmt)]
            ets = {}
            op_box = [None]

            def sc_job(h, k):
                if si == 0 and h == 1 and k == 0:
                    while pending and pending[0][0] == "t2":
                        pending.pop(0)[2]()
                moff, mw = mts[k]
                sc = psc.tile([128, 576], F32, tag="sc", name="sc")
                for co, cw in ((0, 512), (512, 64)):
                    nc.tensor.matmul(
                        sc[0:mw, co:co + cw],
                        p16_sb[s][:, moff:moff + mw],
                        t16[:, 576 * h + co:576 * h + co + cw],
                        start=True, stop=True)
                et = et3.tile([128, 576], BF16, tag="et", name="et")
                nc.scalar.activation(et[0:mw, :], sc[0:mw, :], AFT.Exp,
                                     bias=bias_sb[s][0:mw, :])
                ets[(h, k)] = et

            def op_job(h, k):
                moff, mw = mts[k]
                if k == 0:
                    op_box[0] = pop.tile([128, 576], F32, tag="op", name="op")
                op = op_box[0]
                et = ets.pop((h, k))
                for co, cw in ((0, 512), (512, 64)):
                    nc.tensor.matmul(
                        op[:, co:co + cw],
                        ga[0:mw, 128 * k:128 * k + 128],
                        et[0:mw, co:co + cw],
                        start=(k == 0), stop=(k == nmt - 1))
                if k == nmt - 1:
                    if si == 4 and h == 1:
                        # final half: skip the evac copy, read PSUM directly
                        # to shorten the pre-collective serial chain
                        rc = p2.tile([64, 576], F32, tag="rc", name="rc")
                        nc.vector.reciprocal(rc[:], op[64:128, :])
                        nc.vector.tensor_tensor(
                            outT[:, 640 * h:640 * h + 576],
                            op[0:64, :], rc[:], op=AluOpType.mult)
                    else:
                        opc = p2.tile([128, 576], F32, tag="opc", name="opc")
                        nc.vector.tensor_copy(opc[:], op[:])
                        rc = p2.tile([64, 576], F32, tag="rc", name="rc")
                        nc.vector.reciprocal(rc[:], opc[64:128, :])
                        nc.vector.tensor_tensor(
                            outT[:, 640 * h:640 * h + 576],
                            opc[0:64, :], rc[:], op=AluOpType.mult)
                    # gram units wait on the mult; insert a few slots deep so
                    # their pop doesn't head-block the PE queue
                    pos = min(len(pending), 4)
                    pending[pos:pos] = gram_units(si, s, h, outT,
                                                  gm_boxes[si])

            for j in range(min(3, len(jobs))):
                sc_job(*jobs[j])
            for idx in range(len(jobs)):
                if idx + 3 < len(jobs):
                    sc_job(*jobs[idx + 3])
                op_job(*jobs[idx])
                pop_units()
            # one xbar transpose per scale (both 640-col halves at once)
            xw = sb.tile([128, 10 * 64], F16, tag=f"xw{s}", name=f"xw{s}")
            xw_sb[s] = xw
            nc.sync.dma_start_transpose(
                xw[:].rearrange("p (b c) -> p b c", c=64), outT[:])
            pending.append(gcat_unit(si, G_cat, gm_boxes[si]))
            if si == 3:
                pending += zstat_units(0, 4, G_cat, arin_sb)

        while pending:
            pending.pop(0)[2]()
        # last-scale BN stats: both t-halves in one gx bank (one group, two
        # disjoint 65-col regions) -> single wide DVE pass
        zgl = pgx.tile([128, 512], F32, tag="gx", name="zgl")
        for t in range(2):
            nc.tensor.matmul(zgl[:, 65 * t:65 * t + 65],
                             zwtf_sb[:, 128 * t:128 * t + 128],
                             G_cat[:, 260:325], start=(t == 0), stop=(t == 1),
                             skip_group_check=True)
        zzl = p2.tile([128, 130], F32, tag="zzl", name="zzl")
        zwv = bass.AP(tensor=wb32[:].tensor,
                      offset=wb32[:].offset + Z_ZW65 + 260,
                      ap=[[wb32[:].ap[0][0], 128], [325, 2], [1, 65]])
        nc.vector.tensor_tensor(zzl[:], zgl[:, 0:130], zwv, op=AluOpType.mult)
        s2l = p2.tile([128, 2], F32, tag="s2l", name="s2l")
        nc.vector.tensor_reduce(
            s2l[:], zzl[:].rearrange("p (t c) -> p t c", c=65), AXX,
            AluOpType.add)
        av = bass.AP(tensor=arin_sb[:].tensor, offset=arin_sb[:].offset + 4,
                     ap=[[arin_sb[:].ap[0][0], 128], [10, 2]])
        nc.vector.tensor_copy(av, s2l[:])
        sv1 = bass.AP(tensor=zgl[:].tensor, offset=zgl[:].offset + 64,
                      ap=[[zgl[:].ap[0][0], 128], [65, 2]])
        av1 = bass.AP(tensor=arin_sb[:].tensor, offset=arin_sb[:].offset + 9,
                      ap=[[arin_sb[:].ap[0][0], 128], [10, 2]])
        nc.vector.tensor_copy(av1, sv1)

        # ---------------- stats AllGather ----------------
        arin = dram.tile([128, 20], F16, name="arin_d")
        arout = dram.tile([128 * NCORES, 20], F16, name="arout_d")
        nc.sync.dma_start(arin[:], arin_sb[:])
        nc.gpsimd.collective_compute(
            "AllGather", AluOpType.bypass,
            replica_groups=[list(range(NCORES))],
            ins=[arin.opt()], outs=[arout.opt()])
        gath = sb.tile([128, 160], F16, tag="gath", name="gath")
        src = bass.AP(tensor=arout[:].tensor, offset=arout[:].offset,
                      ap=[[20, 128], [2560, 8], [1, 20]])
        nc.sync.dma_start(gath[:], src)
        # PE pstate warm-keeper: dummy matmuls gated on the gathered stats
        # run during the DVE coefficient chain in the idle op bank, so the
        # a5cat/W/final matmuls start at full clock after the collective
        for wi in range(10):
            wp = pop.tile([128, 576], F32, tag="op", name=f"rw{wi}")
            nc.tensor.matmul(wp[:, 0:160], warm[:, 0:128], gath[:],
                             start=True, stop=True)
        stats = sb.tile([128, 20], F32, tag="stats", name="stats")
        nc.vector.tensor_reduce(
            stats[:],
            bass.AP(tensor=gath[:].tensor, offset=gath[:].offset,
                    ap=[[gath[:].ap[0][0], 128], [1, 20], [20, 8]]),
            AXX, AluOpType.add)

        # ---------------- BN coefficients (both t-halves per op) ----------
        def sv(col0):
            # stats view [128, 2, 5] over the two t-halves at col0 offset
            return bass.AP(tensor=stats[:].tensor,
                           offset=stats[:].offset + col0,
                           ap=[[stats[:].ap[0][0], 128], [10, 2], [1, 5]])

        def pv2(t5):
            return bass.AP(tensor=t5[:].tensor, offset=t5[:].offset,
                           ap=[[t5[:].ap[0][0], 128], [5, 2], [1, 5]])

        a16 = sb.tile([128, 10], F16, tag="a16", name="a16")
        bacc_t = sb.tile([128, 2], F32, tag="bacc", name="bacc")
        mean = p2.tile([128, 10], F32, tag="mean", name="mean")
        nc.vector.tensor_scalar_mul(pv2(mean), sv(5), 1.0 / NPIX)
        m2 = p2.tile([128, 10], F32, tag="m2", name="m2")
        nc.vector.tensor_tensor(m2[:], mean[:], mean[:], op=AluOpType.mult)
        var = p2.tile([128, 10], F32, tag="var", name="var")
        nc.vector.scalar_tensor_tensor(pv2(var), sv(0), 1.0 / NPIX, pv2(m2),
                                       op0=AluOpType.mult,
                                       op1=AluOpType.subtract)
        sq = p2.tile([128, 10], F32, tag="sq", name="sq")
        nc.scalar.activation(sq[:], var[:], AFT.Sqrt,
                             bias=gmp_sb[:, 12:13])
        rinv = p2.tile([128, 10], F32, tag="rinv", name="rinv")
        nc.vector.reciprocal_approx_fast(rinv[:], sq[:])
        af = p2.tile([128, 10], F32, tag="af", name="af")
        nc.vector.tensor_tensor(af[:], rinv[:], gmp_sb[:, 0:10],
                                op=AluOpType.mult)
        nc.vector.tensor_copy(a16[:], af[:])
        tmb = p2.tile([128, 10], F32, tag="tmb", name="tmb")
        nc.vector.tensor_tensor(tmb[:], af[:], mean[:], op=AluOpType.mult)
        tmbr = p2.tile([128, 2], F32, tag="tmbr", name="tmbr")
        nc.vector.tensor_reduce(
            tmbr[:],
            bass.AP(tensor=tmb[:].tensor, offset=tmb[:].offset,
                    ap=[[tmb[:].ap[0][0], 128], [5, 2], [1, 5]]),
            AXX, AluOpType.add)
        nc.vector.tensor_tensor(bacc_t[:], gmp_sb[:, 10:12],
                                tmbr[:], op=AluOpType.subtract)
        # a5cat rows via PE transposes: one accumulation group per bank
        a5cat = sb.tile([1, 1280], F16, tag="a5cat", name="a5cat")
        banks = [(pgx, "gx", 0, 4), (pcv, "cv", 4, 8), (psc, "sc", 8, 10)]
        for pool, tag, i0, i1 in banks:
            atp = pool.tile([128, 512], F32, tag=tag, name="tp")
            for ii in range(i0, i1):
                si, t = ii // 2, ii % 2
                nc.tensor.matmul(
                    atp[0:1, 128 * (ii - i0):128 * (ii - i0) + 128],
                    a16[:, 5 * t + si:5 * t + si + 1],
                    i128_sb[:], start=(ii == i0), stop=(ii == i1 - 1))
            nc.vector.tensor_copy(a5cat[0:1, 128 * i0:128 * i1],
                                  atp[0:1, 0:128 * (i1 - i0)])
        W_sb = []
        for si in range(5):
            abp = (pcv if si % 2 else pgx).tile(
                [128, 512], F32, tag="cv" if si % 2 else "gx", name="ab")
            nc.tensor.matmul(abp[:, 0:256], ones128[:],
                             a5cat[0:1, 256 * si:256 * si + 256],
                             start=True, stop=True)
            W = sb.tile([128, 256], F16, tag=f"W{si}", name=f"W{si}")
            nc.vector.tensor_tensor(W[:], zwt16_sb[:], abp[:, 0:256],
                                    op=AluOpType.mult)
            W_sb.append(W)

        # ---------------- final matmul + store ----------------
        for t in range(2):
            out_sb = sb.tile([128, QL], F32, tag=f"osb{t}", name=f"osb{t}")
            for h in range(2):
                for par in range(2):
                    nb = 5 if par == 0 else 4
                    fp = psc.tile([128, 576], F32, tag="sc", name="sc")
                    for si in range(5):
                        nc.tensor.matmul(
                            fp[:, 0:64 * nb],
                            W_sb[si][64 * par:64 * par + 64,
                                     128 * t:128 * t + 128],
                            xw_sb[SCHED[si]][64 * par:64 * par + 64,
                                             320 * h:320 * h + 64 * nb],
                            start=(si == 0), stop=(si == 4))
                    dst = bass.AP(
                        tensor=out_sb[:].tensor,
                        offset=out_sb[:].offset + 64 * (9 * h + par),
                        ap=[[out_sb[:].ap[0][0], 128], [128, nb], [1, 64]])
                    src = fp[:].rearrange("p (b c) -> p b c", c=64)[:, 0:nb, :]
                    nc.vector.tensor_scalar_add(dst, src,
                                                bacc_t[:, t:t + 1])
                nc.sync.dma_start(
                    out_d[:, QL * t + 576 * h:QL * t + 576 * h + 576],
                    out_sb[:, 576 * h:576 * h + 576])

    nc.compile()
    return nc


def kernel(**inputs):
    f32, f16 = np.float32, np.float16
    persp = np.asarray(inputs['perspective'], dtype=f32)
    t_w = np.asarray(inputs['t_w'], dtype=f32)
    z_w = np.asarray(inputs['z_w'], dtype=f32)
    if 'nc' not in _CACHED:
        _CACHED['nc'] = build()
    nc = _CACHED['nc']

    # local query order: col L = 64*j + q  ->  global n = 36*q + 18*h + j
    Lq = np.arange(QL)
    qv, jv = Lq % 64, Lq // 64
    twt = np.ascontiguousarray(t_w.T)
    zwt = np.ascontiguousarray(z_w.T)

    wb16 = np.zeros((128, W_END), f16)
    wb16[:, W_TWT:W_TWT + 64] = twt[0:128].astype(f16)
    wb16[:, W_TWT + 64:W_TWT + 128] = twt[128:256].astype(f16)
    for s in range(5):
        pw = np.asarray(inputs[f'p{s}_w'], f32).T
        gw = np.asarray(inputs[f'g{s}_w'], f32).T
        for kk in range(KT[s]):
            r0, r1 = 128 * kk, min(128 * kk + 128, CR[s])
            wb16[0:r1 - r0, POFF[s] + 64 * kk:POFF[s] + 64 * kk + 64] = \
                pw[r0:r1].astype(f16)
            wb16[0:r1 - r0, GOFF[s] + 64 * kk:GOFF[s] + 64 * kk + 64] = \
                gw[r0:r1].astype(f16)
    wb16[:, W_ZWT:W_ZWT + 256] = np.concatenate([zwt, zwt], axis=0).astype(f16)
    wb16[:, W_I128:W_I128 + 128] = np.eye(128, dtype=f16)

    wb32 = np.zeros((128, Z_END), f32)
    wb32[0:64, Z_ZWTF:Z_ZWTF + 256] = zwt
    for t in range(2):
        for si in range(5):
            wb32[:, Z_ZW65 + 325 * t + 65 * si:Z_ZW65 + 325 * t + 65 * si + 64] = \
                z_w[128 * t:128 * t + 128, :]
            wb32[:, Z_GMP + 5 * t + si] = np.asarray(
                inputs[f'bn{SCHED[si]}_g'], f32)[128 * t:128 * t + 128]
        wb32[:, Z_GMP + 10 + t] = sum(np.asarray(inputs[f'bn{s}_b'], f32)
                                      for s in range(5))[128 * t:128 * t + 128]
    wb32[:, Z_GMP + 12] = EPS

    def pack_resp(i, s):
        rs = np.asarray(inputs[f'response{s}'], f32)[i].reshape(CR[s], MS[s])
        rt = np.zeros((min(CR[s], 128), KT[s] * MS[s]), f16)
        for kk in range(KT[s]):
            r0, r1 = 128 * kk, min(128 * kk + 128, CR[s])
            rt[0:r1 - r0, MS[s] * kk:MS[s] * kk + MS[s]] = rs[r0:r1].astype(f16)
        return rt

    in_maps = []
    for i in range(4):
        for h in range(2):
            nglob = 36 * qv + 18 * h + jv
            pi = persp[i].reshape(CP, 2304)[:, nglob]
            p16 = np.zeros((128, 2 * QL), f16)
            p16[:, 0:QL] = pi[0:128].astype(f16)
            p16[:, QL:] = pi[128:256].astype(f16)
            m = {"persp": p16, "wb16": wb16, "wb32": wb32,
                 "resp0": pack_resp(i, 0), "resp1": pack_resp(i, 1),
                 "r234": np.concatenate(
                     [pack_resp(i, 2), pack_resp(i, 3), pack_resp(i, 4)],
                     axis=1)}
            in_maps.append(m)
    res = bass_utils.run_bass_kernel_spmd(nc, in_maps,
                                          core_ids=list(range(NCORES)))
    _CACHED['res'] = res
    out = np.zeros((4, CP, 2304), np.float32)
    for i in range(4):
        for h in range(2):
            o = res.results[i * 2 + h]["out"]
            full = np.concatenate([o[:, 0:QL], o[:, QL:]], axis=0)
            out[i][:, QL * h:QL * h + QL] = full
    return out.reshape(4, CP, 48, 48)


if __name__ == "__main__":
    from concourse.timeline_sim import TimelineSim
    nc = build()
    tl = TimelineSim(nc, trace=False)
    print(f"TimelineSim: {tl.simulate():.0f} ns")
